# revision 14
# baseline (speedup 1.0000x reference)
"""Self-contained Trainium2 Bass kernel for nn_CharModel (dense transformer
forward: embed -> single-head causal attention -> vocab projection).

Distribution over 8 NeuronCores:
  - sequence-parallel QKV: core c computes Q/K/V only for its own 512 tokens;
    K^T is exchanged with one fp8 AllGather, V with two chunked fp8 AllGathers
  - sequence-parallel attention rows (fp8 DoubleRow scores + AV)
  - token-parallel logits: each core computes the FULL vocab row block for its
    own 512 tokens, streaming Wp (fp8) from HBM strip by strip; no output
    collective needed -- host just concatenates row blocks
Host pre-casts: E/Wq/Wk/Wv to bf16, Wp.T*64 to fp8e4, and folds bv@Wp.T+bp
into a single f32 bias row. Logits are written bf16 (host upcasts).
"""
import numpy as np
import ml_dtypes

import concourse.bass as bass
import concourse.mybir as mybir
import concourse.tile as tile
from concourse import bacc
from concourse.bass_utils import run_bass_kernel_spmd
from concourse.masks import make_identity

P = 128
N_TOK = 4096
D = 1024
VOCAB = 50257
NC = 8
NSTRIP = 99  # vocab strips of 512
VPAD = NSTRIP * 512  # 50688
OWN = N_TOK // NC  # 512 own tokens
IBLK = OWN // P  # 4 own row-blocks
KT = D // P  # 8 contraction tiles
OT = D // P  # 8 output-feature tiles
NPAIR = 16  # key-tile pairs per half (tb-major over ranks)
SCALE = 1.0 / 32.0  # 1/sqrt(D)

F32 = mybir.dt.float32
BF16 = mybir.dt.bfloat16
FP8 = mybir.dt.float8e4
I32 = mybir.dt.int32
WP_SCALE = 64.0
OUT_SCALE = 256.0
Q_SCALE = 64.0
K_SCALE = 16.0
EXP_SCALE = SCALE / (Q_SCALE * K_SCALE)  # exp((K*16)(Q*64)/32768) = exp(QK/32)

KV_K = P * OT * 512  # kT elements per rank in the k exchange buffer
KV_V = P * D  # one V tile (128 tokens x 1024)
DR = mybir.MatmulPerfMode.DoubleRow


def build(nc: bass.Bass):
    qtok = nc.dram_tensor("qtok", [OWN], I32, kind="ExternalInput")
    E = nc.dram_tensor("E", [VOCAB, D], BF16, kind="ExternalInput")
    WqT = nc.dram_tensor("WqT", [D, D], BF16, kind="ExternalInput")
    WkT = nc.dram_tensor("WkT", [D, D], BF16, kind="ExternalInput")
    WvT = nc.dram_tensor("WvT", [D, D], BF16, kind="ExternalInput")
    bq = nc.dram_tensor("bq", [D], F32, kind="ExternalInput")
    bk = nc.dram_tensor("bk", [D], F32, kind="ExternalInput")
    Wp8 = nc.dram_tensor("Wp8", [D, VPAD], FP8, kind="ExternalInput")
    # scol[h, j, p]: causal mask thresholds in the tb-major key order
    scol = nc.dram_tensor("scol", [2 * 2 * NPAIR * P], F32, kind="ExternalInput")
    logits = nc.dram_tensor("logits", [OWN, VPAD], BF16, kind="ExternalOutput")

    with tile.TileContext(nc) as tc:
        with (
            tc.tile_pool(name="const", bufs=1) as const,
            tc.tile_pool(name="dram", bufs=1, space="DRAM") as dram,
        ):
            ident = const.tile([P, P], BF16)
            make_identity(nc, ident[:])

            bq_t = const.tile([P, OT], F32)
            nc.sync.dma_start(bq_t[:], bq.ap().rearrange("(ot p) -> p ot", p=P))
            bk_t = const.tile([P, OT], F32)
            nc.sync.dma_start(bk_t[:], bk.ap().rearrange("(ot p) -> p ot", p=P))

            scol_sb = const.tile([P, 2, 2 * NPAIR], F32)
            nc.sync.dma_start(
                scol_sb[:], scol.ap().rearrange("(h j p) -> p h j", p=P, h=2)
            )
            ones_col = const.tile([P, 1], BF16)
            nc.vector.memset(ones_col[:], 1.0)
            ones_row = const.tile([1, P], BF16)
            nc.vector.memset(ones_row[:], 1.0)

            jidx0 = const.tile([P, 512], F32)

            qtok_sb = const.tile([P, OWN // P], I32)
            nc.sync.dma_start(qtok_sb[:], qtok.ap().rearrange("(g p) -> p g", p=P))

            # DRAM scratch for collectives
            k_send = dram.tile([KV_K], FP8)
            v_send_a = dram.tile([2 * KV_V], FP8)
            v_send_b = dram.tile([2 * KV_V], FP8)
            kvg_k = dram.tile([NC, KV_K], FP8, addr_space="Shared")
            kvg_va = dram.tile([NC, 2 * KV_V], FP8, addr_space="Shared")
            kvg_vb = dram.tile([NC, 2 * KV_V], FP8, addr_space="Shared")

            # ---------------- phase QKV: own tokens only --------------------
            qT_pool = tc.alloc_tile_pool(name="qT_keep", bufs=1)
            qT8 = qT_pool.tile([P, OT, OWN], FP8)
            kT_pool = tc.alloc_tile_pool(name="kT_keep", bufs=1)
            kT_r = [
                kT_pool.tile([P, OT, 512], FP8, name=f"kTr{r}")
                for r in range(NC)
            ]
            oT_pool = tc.alloc_tile_pool(name="oT_keep", bufs=1)
            oT_keep = oT_pool.tile([P, KT, OWN], FP8)
            with (
                tc.tile_pool(name="sbw", bufs=1) as sbw,
                tc.tile_pool(name="sbq", bufs=2) as sbq,
                tc.tile_pool(name="psq_tr", bufs=2, space="PSUM") as psq_tr,
                tc.tile_pool(name="psq_pp", bufs=2, space="PSUM") as psq_pp,
                tc.tile_pool(name="psq_pv", bufs=2, space="PSUM") as psq_pv,
            ):
                ji = sbw.tile([P, 512], I32, tag="ji")
                nc.gpsimd.iota(ji[:], pattern=[[1, 512]], base=0, channel_multiplier=0)
                nc.vector.tensor_copy(out=jidx0[:], in_=ji[:])

                wk_b = sbw.tile([P, KT, D], BF16, tag="wk")
                nc.sync.dma_start(
                    wk_b[:], WkT.ap().rearrange("(kt p) o -> p kt o", p=P)
                )

                # gather own embeddings + transpose -> xT [P, KT, OWN]
                xT = sbw.tile([P, KT, OWN], BF16, tag="xT")
                for g in range(IBLK):
                    xg = sbq.tile([P, D], BF16, tag="xg")
                    nc.gpsimd.indirect_dma_start(
                        out=xg[:],
                        out_offset=None,
                        in_=E.ap(),
                        in_offset=bass.IndirectOffsetOnAxis(
                            ap=qtok_sb[:, g : g + 1], axis=0
                        ),
                    )
                    for kt in range(KT):
                        pst = psq_tr.tile([P, P], BF16, tag="ptr")
                        nc.tensor.transpose(
                            pst[:], xg[:, kt * P : (kt + 1) * P], ident[:]
                        )
                        nc.vector.tensor_copy(
                            out=xT[:, kt, g * P : (g + 1) * P], in_=pst[:]
                        )

                wq_b = sbw.tile([P, KT, D], BF16, tag="wq")
                nc.sync.dma_start(
                    wq_b[:], WqT.ap().rearrange("(kt p) o -> p kt o", p=P)
                )
                wv_b = sbw.tile([P, KT, D], BF16, tag="wv")
                nc.sync.dma_start(
                    wv_b[:], WvT.ap().rearrange("(kt p) o -> p kt o", p=P)
                )

                # K^T for own tokens -> k_send, AllGather
                kT_own = sbw.tile([P, OT, OWN], FP8, tag="kT_own")
                for ot in range(OT):
                    pk = psq_pp.tile([P, OWN], F32, tag="pp")
                    for kt in range(KT):
                        nc.tensor.matmul(
                            pk[:],
                            lhsT=wk_b[:, kt, ot * P : (ot + 1) * P],
                            rhs=xT[:, kt, :],
                            start=(kt == 0),
                            stop=(kt == KT - 1),
                        )
                    nc.vector.tensor_scalar(
                        out=kT_own[:, ot, :],
                        in0=pk[:],
                        scalar1=bk_t[:, ot : ot + 1],
                        scalar2=K_SCALE,
                        op0=mybir.AluOpType.add,
                        op1=mybir.AluOpType.mult,
                    )
                nc.sync.dma_start(
                    k_send[:].rearrange("(p ot j) -> p ot j", p=P, ot=OT),
                    kT_own[:],
                )
                nc.gpsimd.collective_compute(
                    "AllGather",
                    mybir.AluOpType.bypass,
                    replica_groups=[list(range(NC))],
                    ins=[k_send.opt()],
                    outs=[kvg_k.opt()],
                )

                # V for own tokens (NO bias - folded into host bias_row),
                # two chunks of 2 tiles, each AllGathered as it completes
                for tb in range(IBLK):
                    pv = psq_pv.tile([P, D], F32, tag="pv")
                    for half in range(2):
                        for kt in range(KT):
                            nc.tensor.matmul(
                                pv[:, half * 512 : (half + 1) * 512],
                                lhsT=xT[:, kt, tb * P : (tb + 1) * P],
                                rhs=wv_b[:, kt, half * 512 : (half + 1) * 512],
                                start=(kt == 0),
                                stop=(kt == KT - 1),
                            )
                    ve = sbq.tile([P, D], FP8, tag="ve")
                    nc.vector.tensor_scalar(
                        out=ve[:],
                        in0=pv[:],
                        scalar1=K_SCALE,
                        scalar2=None,
                        op0=mybir.AluOpType.mult,
                    )
                    dst = v_send_a if tb < 2 else v_send_b
                    nc.sync.dma_start(
                        dst[(tb % 2) * KV_V : (tb % 2 + 1) * KV_V].rearrange(
                            "(p d) -> p d", p=P
                        ),
                        ve[:],
                    )
                    if tb == 1:
                        nc.gpsimd.collective_compute(
                            "AllGather",
                            mybir.AluOpType.bypass,
                            replica_groups=[list(range(NC))],
                            ins=[v_send_a.opt()],
                            outs=[kvg_va.opt()],
                        )
                    elif tb == 3:
                        nc.gpsimd.collective_compute(
                            "AllGather",
                            mybir.AluOpType.bypass,
                            replica_groups=[list(range(NC))],
                            ins=[v_send_b.opt()],
                            outs=[kvg_vb.opt()],
                        )

                # Q^T scaled to fp8 (x64), kept in SBUF
                for ot in range(OT):
                    pp = psq_pp.tile([P, OWN], F32, tag="pp")
                    for kt in range(KT):
                        nc.tensor.matmul(
                            pp[:],
                            lhsT=wq_b[:, kt, ot * P : (ot + 1) * P],
                            rhs=xT[:, kt, :],
                            start=(kt == 0),
                            stop=(kt == KT - 1),
                        )
                    nc.vector.tensor_scalar(
                        out=qT8[:, ot, :],
                        in0=pp[:],
                        scalar1=bq_t[:, ot : ot + 1],
                        scalar2=Q_SCALE,
                        op0=mybir.AluOpType.add,
                        op1=mybir.AluOpType.mult,
                    )

            # load gathered K^T per rank (scores on rank r wait only on its own)
            for r in range(NC):
                nc.sync.dma_start(
                    kT_r[r][:],
                    kvg_k[r, :].rearrange("(p ot j) -> p ot j", p=P, ot=OT),
                )

            # ---------------- attention (fp8 DoubleRow, tb-major keys) ------
            for half in range(2):
                with (
                    tc.tile_pool(name=f"sbat{half}", bufs=2) as sbat,
                    tc.tile_pool(name=f"ps_sc{half}", bufs=2, space="PSUM") as ps_sc,
                    tc.tile_pool(name=f"ps_av{half}", bufs=1, space="PSUM") as ps_av,
                    tc.tile_pool(name=f"ps_sum{half}", bufs=1, space="PSUM") as ps_sum,
                    tc.tile_pool(name=f"ps_bc{half}", bufs=1, space="PSUM") as ps_bc,
                ):
                    pav = ps_av.tile([P, KT, 256], F32, tag="av")
                    srow = ps_sum.tile([1, 256], F32, tag="srow")
                    for m in range(NPAIR):
                        r = m % NC
                        tb2 = (m // NC) * 2  # first key-tile of the pair
                        vj2 = sbat.tile([P, 2, D], FP8, tag="vj2")
                        src = kvg_va if m < NC else kvg_vb
                        nc.sync.dma_start(
                            vj2[:],
                            src[r, :].rearrange("(t p d) -> p t d", t=2, p=P),
                        )
                        astr2 = sbat.tile([P, 2, 256], FP8, tag="astr2")
                        for i in range(2):
                            t = tb2 + i
                            ps = ps_sc.tile([P, 256], F32, tag="sc")
                            for k2 in range(KT // 2):
                                nc.tensor.matmul(
                                    ps[:],
                                    lhsT=kT_r[r][
                                        :, 2 * k2 : 2 * k2 + 2,
                                        t * P : (t + 1) * P,
                                    ],
                                    rhs=qT8[
                                        :, 2 * k2 : 2 * k2 + 2,
                                        half * 256 : (half + 1) * 256,
                                    ],
                                    start=(k2 == 0),
                                    stop=(k2 == KT // 2 - 1),
                                    perf_mode=DR,
                                )
                            nc.scalar.activation(
                                astr2[:, i, :],
                                ps[:],
                                mybir.ActivationFunctionType.Exp,
                                scale=EXP_SCALE,
                            )
                            # causal mask: keep exp where q_local >= scol
                            nc.vector.scalar_tensor_tensor(
                                out=astr2[:, i, :],
                                in0=jidx0[:, 0:256],
                                scalar=scol_sb[:, half, 2 * m + i : 2 * m + i + 1],
                                in1=astr2[:, i, :],
                                op0=mybir.AluOpType.is_ge,
                                op1=mybir.AluOpType.mult,
                            )
                            nc.tensor.matmul(
                                srow[:],
                                lhsT=ones_col[:],
                                rhs=astr2[:, i, :],
                                start=(m == 0 and i == 0),
                                stop=(m == NPAIR - 1 and i == 1),
                            )
                        for dc in range(KT):
                            nc.tensor.matmul(
                                pav[:, dc, :],
                                lhsT=vj2[:, :, dc * P : (dc + 1) * P],
                                rhs=astr2[:],
                                start=(m == 0),
                                stop=(m == NPAIR - 1),
                                perf_mode=DR,
                            )
                    # normalize: oT = pav * (OUT_SCALE/16) / rowsum, in fp8
                    rr = sbat.tile([1, 256], F32, tag="rr")
                    nc.vector.reciprocal(rr[:], srow[:])
                    rr2 = sbat.tile([1, 256], BF16, tag="rr2")
                    nc.vector.tensor_scalar(
                        out=rr2[:],
                        in0=rr[:],
                        scalar1=OUT_SCALE / K_SCALE,
                        scalar2=None,
                        op0=mybir.AluOpType.mult,
                    )
                    bc = ps_bc.tile([P, 256], F32, tag="bc")
                    nc.tensor.matmul(
                        bc[:], lhsT=ones_row[:], rhs=rr2[:], start=True, stop=True
                    )
                    bc_sb = sbat.tile([P, 256], F32, tag="bc_sb")
                    nc.vector.tensor_copy(out=bc_sb[:], in_=bc[:])
                    for dc in range(KT):
                        nc.vector.scalar_tensor_tensor(
                            out=oT_keep[:, dc, half * 256 : (half + 1) * 256],
                            in0=pav[:, dc, :],
                            scalar=1.0,
                            in1=bc_sb[:],
                            op0=mybir.AluOpType.mult,
                            op1=mybir.AluOpType.mult,
                        )

            # ---------------- logits: own 512 tokens x full vocab -----------
            # Wp streamed from HBM strip by strip; each strip reused for the
            # 4 own row-blocks.
            with (
                tc.tile_pool(name="sbwp", bufs=3) as sbwp,
                tc.tile_pool(name="sblo", bufs=8) as sblo,
                tc.tile_pool(name="pslg", bufs=7, space="PSUM") as pslg,
            ):
                for s in range(NSTRIP):
                    v0 = s * 512
                    wps = sbwp.tile([P, KT, 512], FP8, tag="wps")
                    nc.sync.dma_start(
                        wps[:],
                        Wp8.ap()[:, v0 : v0 + 512].rearrange(
                            "(kt p) v -> p kt v", p=P
                        ),
                    )
                    for q in range(IBLK):
                        pl = pslg.tile([P, 512], F32, tag="lg")
                        for k2 in range(KT // 2):
                            nc.tensor.matmul(
                                pl[:],
                                lhsT=oT_keep[
                                    :, 2 * k2 : 2 * k2 + 2, q * P : (q + 1) * P
                                ],
                                rhs=wps[:, 2 * k2 : 2 * k2 + 2, :],
                                start=(k2 == 0),
                                stop=(k2 == KT // 2 - 1),
                                perf_mode=DR,
                            )
                        lo = sblo.tile([P, 512], BF16, tag="lo")
                        nc.scalar.activation(
                            lo[:, :256], pl[:, :256],
                            mybir.ActivationFunctionType.Copy,
                        )
                        nc.vector.tensor_copy(out=lo[:, 256:], in_=pl[:, 256:])
                        nc.gpsimd.dma_start(
                            logits.ap()[q * P : (q + 1) * P, v0 : v0 + 512],
                            lo[:],
                        )
            oT_pool.release()
            kT_pool.release()
            qT_pool.release()
    return nc


def _prep_inputs(inputs):
    """Host-side shard prep: slicing, transposes, padding, dtype pre-casts."""
    tokens = np.ascontiguousarray(np.asarray(inputs["tokens"]).astype(np.int32))
    E16 = np.asarray(inputs["E"], np.float32).astype(ml_dtypes.bfloat16)
    WqT = np.ascontiguousarray(
        np.asarray(inputs["Wq"], np.float32).T.astype(ml_dtypes.bfloat16)
    )
    WkT = np.ascontiguousarray(
        np.asarray(inputs["Wk"], np.float32).T.astype(ml_dtypes.bfloat16)
    )
    WvT = np.ascontiguousarray(
        np.asarray(inputs["Wv"], np.float32).T.astype(ml_dtypes.bfloat16)
    )
    Wp = np.asarray(inputs["Wp"], np.float32)
    bv = np.asarray(inputs["bv"], np.float32)
    WpT_pad = np.zeros((D, VPAD), np.float32)
    WpT_pad[:, :VOCAB] = Wp.T
    Wp8_full = np.ascontiguousarray(
        (WpT_pad * WP_SCALE).astype(ml_dtypes.float8_e4m3)
    )
    bias_full = np.zeros((VPAD,), np.float32)
    bias_full[:VOCAB] = np.asarray(inputs["bp"], np.float32) + Wp @ bv

    in_maps = []
    p_idx = np.arange(P, dtype=np.float32)
    for c in range(NC):
        # tb-major key order: pair m sub i -> key tile of rank r=m%8,
        # local tile t = (m//8)*2 + i; key base token = r*512 + t*128
        scol = np.empty((2, 2 * NPAIR, P), np.float32)
        for h in range(2):
            for m in range(NPAIR):
                r = m % NC
                for i in range(2):
                    t = (m // NC) * 2 + i
                    base = r * 512 + t * 128
                    scol[h, 2 * m + i, :] = (
                        base + p_idx - (c * 512.0 + h * 256.0)
                    )
        in_maps.append(
            {
                "qtok": np.ascontiguousarray(tokens[c * OWN : (c + 1) * OWN]),
                "E": E16,
                "WqT": WqT,
                "WkT": WkT,
                "WvT": WvT,
                "bq": np.asarray(inputs["bq"], np.float32),
                "bk": np.asarray(inputs["bk"], np.float32),
                "Wp8": Wp8_full,
                "scol": np.ascontiguousarray(scol.ravel()),
            }
        )
    return in_maps, bias_full


def _run(inputs, trace=False):
    nc = bacc.Bacc(trn_type="TRN2", num_devices=NC, debug=False)
    build(nc)
    nc.compile()
    in_maps, bias_full = _prep_inputs(inputs)
    res = run_bass_kernel_spmd(
        nc, in_maps, core_ids=list(range(NC)), trace=trace
    )
    dq = 1.0 / (WP_SCALE * OUT_SCALE)
    out = np.concatenate(
        [
            np.asarray(res.results[c]["logits"], np.float32) * dq
            + bias_full[None, :]
            for c in range(NC)
        ],
        axis=0,
    )[:, :VOCAB]
    return out, res


def kernel(**inputs) -> np.ndarray:
    out, _ = _run(inputs, trace=False)
    return out


# revision 19
# speedup vs baseline: 1.1071x; 1.1071x over previous
"""Self-contained Trainium2 Bass kernel for nn_CharModel (dense transformer
forward: embed -> single-head causal attention -> vocab projection).

Distribution over 8 NeuronCores:
  - sequence-parallel QKV: core c computes Q/K/V only for its own 512 tokens;
    K^T is exchanged with one fp8 AllGather, V with two chunked fp8 AllGathers
  - sequence-parallel attention rows (fp8 DoubleRow scores + AV)
  - token-parallel logits: each core computes the FULL vocab row block for its
    own 512 tokens, streaming Wp (fp8) from HBM strip by strip; no output
    collective needed -- host just concatenates row blocks
Host pre-casts: E/Wq/Wk/Wv to bf16, Wp.T*64 to fp8e4, and folds bv@Wp.T+bp
into a single f32 bias row. Logits are written bf16 (host upcasts).
"""
import numpy as np
import ml_dtypes

import concourse.bass as bass
import concourse.mybir as mybir
import concourse.tile as tile
from concourse import bacc
from concourse.bass_utils import run_bass_kernel_spmd
from concourse.masks import make_identity

P = 128
N_TOK = 4096
D = 1024
VOCAB = 50257
NC = 8
NSTRIP = 99  # vocab strips of 512
VPAD = NSTRIP * 512  # 50688
OWN = N_TOK // NC  # 512 own tokens
IBLK = OWN // P  # 4 own row-blocks
KT = D // P  # 8 contraction tiles
OT = D // P  # 8 output-feature tiles
NPAIR = 16  # key-tile pairs per half (tb-major over ranks)
SCALE = 1.0 / 32.0  # 1/sqrt(D)

F32 = mybir.dt.float32
BF16 = mybir.dt.bfloat16
FP8 = mybir.dt.float8e4
I32 = mybir.dt.int32
WP_SCALE = 64.0
OUT_SCALE = 256.0
Q_SCALE = 64.0
K_SCALE = 16.0
EXP_SCALE = SCALE / (Q_SCALE * K_SCALE)  # exp((K*16)(Q*64)/32768) = exp(QK/32)

KV_K = P * OT * 512  # kT elements per rank in the k exchange buffer
KV_V = P * D  # one V tile (128 tokens x 1024)
DR = mybir.MatmulPerfMode.DoubleRow


def build(nc: bass.Bass):
    qtok = nc.dram_tensor("qtok", [OWN], I32, kind="ExternalInput")
    E = nc.dram_tensor("E", [VOCAB, D], BF16, kind="ExternalInput")
    WqT = nc.dram_tensor("WqT", [D, D], BF16, kind="ExternalInput")
    WkT = nc.dram_tensor("WkT", [D, D], BF16, kind="ExternalInput")
    WvT = nc.dram_tensor("WvT", [D, D], BF16, kind="ExternalInput")
    bq = nc.dram_tensor("bq", [D], F32, kind="ExternalInput")
    bk = nc.dram_tensor("bk", [D], F32, kind="ExternalInput")
    Wp8 = nc.dram_tensor("Wp8", [D, VPAD], FP8, kind="ExternalInput")
    # scol[h, j, p]: causal mask thresholds in the tb-major key order
    scol = nc.dram_tensor("scol", [2 * 2 * NPAIR * P], F32, kind="ExternalInput")
    logits = nc.dram_tensor("logits", [OWN, VPAD], BF16, kind="ExternalOutput")

    with tile.TileContext(nc) as tc:
        with (
            tc.tile_pool(name="const", bufs=1) as const,
            tc.tile_pool(name="dram", bufs=1, space="DRAM") as dram,
        ):
            ident = const.tile([P, P], BF16)
            make_identity(nc, ident[:])

            qtok_sb = const.tile([P, OWN // P], I32)
            nc.sync.dma_start(qtok_sb[:], qtok.ap().rearrange("(g p) -> p g", p=P))

            bq_t = const.tile([P, OT], F32)
            nc.sync.dma_start(bq_t[:], bq.ap().rearrange("(ot p) -> p ot", p=P))
            bk_t = const.tile([P, OT], F32)
            nc.sync.dma_start(bk_t[:], bk.ap().rearrange("(ot p) -> p ot", p=P))

            scol_sb = const.tile([P, 2, 2 * NPAIR], F32)
            nc.sync.dma_start(
                scol_sb[:], scol.ap().rearrange("(h j p) -> p h j", p=P, h=2)
            )
            ones_col = const.tile([P, 1], BF16)
            nc.vector.memset(ones_col[:], 1.0)
            ones_row = const.tile([1, P], BF16)
            nc.vector.memset(ones_row[:], 1.0)

            jidx0 = const.tile([P, 512], F32)

            # DRAM scratch for collectives
            k_send = dram.tile([KV_K], FP8)
            v_send = dram.tile([4 * KV_V], FP8)
            kvg_k = dram.tile([NC, KV_K], FP8, addr_space="Shared")
            kvg_v = dram.tile([NC, 4 * KV_V], FP8, addr_space="Shared")

            # ---------------- phase QKV: own tokens only --------------------
            qT_pool = tc.alloc_tile_pool(name="qT_keep", bufs=1)
            qT8 = qT_pool.tile([P, OT, OWN], FP8)
            kT_pool = tc.alloc_tile_pool(name="kT_keep", bufs=1)
            kT_r = [
                kT_pool.tile([P, OT, 512], FP8, name=f"kTr{r}")
                for r in range(NC)
            ]
            oT_pool = tc.alloc_tile_pool(name="oT_keep", bufs=1)
            oT_keep = oT_pool.tile([P, KT, OWN], FP8)
            with (
                tc.tile_pool(name="sbw", bufs=1) as sbw,
                tc.tile_pool(name="sbq", bufs=2) as sbq,
                tc.tile_pool(name="psq_tr", bufs=2, space="PSUM") as psq_tr,
                tc.tile_pool(name="psq_pp", bufs=2, space="PSUM") as psq_pp,
                tc.tile_pool(name="psq_pv", bufs=2, space="PSUM") as psq_pv,
            ):
                ji = sbw.tile([P, 512], I32, tag="ji")
                nc.gpsimd.iota(ji[:], pattern=[[1, 512]], base=0, channel_multiplier=0)
                nc.vector.tensor_copy(out=jidx0[:], in_=ji[:])

                wk_b = sbw.tile([P, KT, D], BF16, tag="wk")
                nc.sync.dma_start(
                    wk_b[:], WkT.ap().rearrange("(kt p) o -> p kt o", p=P)
                )

                # gather own embeddings + transpose -> xT [P, KT, OWN]
                xT = sbw.tile([P, KT, OWN], BF16, tag="xT")
                for g in range(IBLK):
                    xg = sbq.tile([P, D], BF16, tag="xg")
                    nc.gpsimd.indirect_dma_start(
                        out=xg[:],
                        out_offset=None,
                        in_=E.ap(),
                        in_offset=bass.IndirectOffsetOnAxis(
                            ap=qtok_sb[:, g : g + 1], axis=0
                        ),
                    )
                    for kt in range(KT):
                        pst = psq_tr.tile([P, P], BF16, tag="ptr")
                        nc.tensor.transpose(
                            pst[:], xg[:, kt * P : (kt + 1) * P], ident[:]
                        )
                        nc.vector.tensor_copy(
                            out=xT[:, kt, g * P : (g + 1) * P], in_=pst[:]
                        )

                wq_b = sbw.tile([P, KT, D], BF16, tag="wq")
                nc.sync.dma_start(
                    wq_b[:], WqT.ap().rearrange("(kt p) o -> p kt o", p=P)
                )
                wv_b = sbw.tile([P, KT, D], BF16, tag="wv")
                nc.sync.dma_start(
                    wv_b[:], WvT.ap().rearrange("(kt p) o -> p kt o", p=P)
                )

                # K^T for own tokens -> k_send, AllGather
                kT_own = sbw.tile([P, OT, OWN], FP8, tag="kT_own")
                for ot in range(OT):
                    pk = psq_pp.tile([P, OWN], F32, tag="pp")
                    for kt in range(KT):
                        nc.tensor.matmul(
                            pk[:],
                            lhsT=wk_b[:, kt, ot * P : (ot + 1) * P],
                            rhs=xT[:, kt, :],
                            start=(kt == 0),
                            stop=(kt == KT - 1),
                        )
                    nc.vector.tensor_scalar(
                        out=kT_own[:, ot, :],
                        in0=pk[:],
                        scalar1=bk_t[:, ot : ot + 1],
                        scalar2=K_SCALE,
                        op0=mybir.AluOpType.add,
                        op1=mybir.AluOpType.mult,
                    )
                nc.sync.dma_start(
                    k_send[:].rearrange("(p ot j) -> p ot j", p=P, ot=OT),
                    kT_own[:],
                )
                nc.gpsimd.collective_compute(
                    "AllGather",
                    mybir.AluOpType.bypass,
                    replica_groups=[list(range(NC))],
                    ins=[k_send.opt()],
                    outs=[kvg_k.opt()],
                )

                # V for own tokens (NO bias - folded into host bias_row),
                # two chunks of 2 tiles, each AllGathered as it completes
                for tb in range(IBLK):
                    pv = psq_pv.tile([P, D], F32, tag="pv")
                    for half in range(2):
                        for kt in range(KT):
                            nc.tensor.matmul(
                                pv[:, half * 512 : (half + 1) * 512],
                                lhsT=xT[:, kt, tb * P : (tb + 1) * P],
                                rhs=wv_b[:, kt, half * 512 : (half + 1) * 512],
                                start=(kt == 0),
                                stop=(kt == KT - 1),
                            )
                    ve = sbq.tile([P, D], FP8, tag="ve")
                    nc.vector.tensor_scalar(
                        out=ve[:],
                        in0=pv[:],
                        scalar1=K_SCALE,
                        scalar2=None,
                        op0=mybir.AluOpType.mult,
                    )
                    nc.sync.dma_start(
                        v_send[tb * KV_V : (tb + 1) * KV_V].rearrange(
                            "(p d) -> p d", p=P
                        ),
                        ve[:],
                    )
                    if tb == 3:
                        nc.gpsimd.collective_compute(
                            "AllGather",
                            mybir.AluOpType.bypass,
                            replica_groups=[list(range(NC))],
                            ins=[v_send.opt()],
                            outs=[kvg_v.opt()],
                        )

                # Q^T scaled to fp8 (x64), kept in SBUF
                for ot in range(OT):
                    pp = psq_pp.tile([P, OWN], F32, tag="pp")
                    for kt in range(KT):
                        nc.tensor.matmul(
                            pp[:],
                            lhsT=wq_b[:, kt, ot * P : (ot + 1) * P],
                            rhs=xT[:, kt, :],
                            start=(kt == 0),
                            stop=(kt == KT - 1),
                        )
                    nc.vector.tensor_scalar(
                        out=qT8[:, ot, :],
                        in0=pp[:],
                        scalar1=bq_t[:, ot : ot + 1],
                        scalar2=Q_SCALE,
                        op0=mybir.AluOpType.add,
                        op1=mybir.AluOpType.mult,
                    )

            # load gathered K^T per rank (scores on rank r wait only on its own)
            for r in range(NC):
                nc.sync.dma_start(
                    kT_r[r][:],
                    kvg_k[r, :].rearrange("(p ot j) -> p ot j", p=P, ot=OT),
                )

            # ---------------- attention (fp8 DoubleRow, tb-major keys) ------
            for half in range(2):
                with (
                    tc.tile_pool(name=f"sbat{half}", bufs=3) as sbat,
                    tc.tile_pool(name=f"ps_sc{half}", bufs=2, space="PSUM") as ps_sc,
                    tc.tile_pool(name=f"ps_av{half}", bufs=1, space="PSUM") as ps_av,
                    tc.tile_pool(name=f"ps_sum{half}", bufs=1, space="PSUM") as ps_sum,
                    tc.tile_pool(name=f"ps_bc{half}", bufs=1, space="PSUM") as ps_bc,
                ):
                    pav = ps_av.tile([P, KT, 256], F32, tag="av")
                    srow = ps_sum.tile([1, 256], F32, tag="srow")
                    for m in range(NPAIR):
                        r = m % NC
                        tb2 = (m // NC) * 2  # first key-tile of the pair
                        vj2 = sbat.tile([P, 2, D], FP8, tag="vj2")
                        nc.sync.dma_start(
                            vj2[:],
                            kvg_v[
                                r, tb2 * KV_V : (tb2 + 2) * KV_V
                            ].rearrange("(t p d) -> p t d", t=2, p=P),
                        )
                        astr2 = sbat.tile([P, 2, 256], FP8, tag="astr2")
                        for i in range(2):
                            t = tb2 + i
                            ps = ps_sc.tile([P, 256], F32, tag="sc")
                            for k2 in range(KT // 2):
                                nc.tensor.matmul(
                                    ps[:],
                                    lhsT=kT_r[r][
                                        :, 2 * k2 : 2 * k2 + 2,
                                        t * P : (t + 1) * P,
                                    ],
                                    rhs=qT8[
                                        :, 2 * k2 : 2 * k2 + 2,
                                        half * 256 : (half + 1) * 256,
                                    ],
                                    start=(k2 == 0),
                                    stop=(k2 == KT // 2 - 1),
                                    perf_mode=DR,
                                )
                            nc.scalar.activation(
                                astr2[:, i, :],
                                ps[:],
                                mybir.ActivationFunctionType.Exp,
                                scale=EXP_SCALE,
                            )
                            # causal mask: keep exp where q_local >= scol
                            nc.vector.scalar_tensor_tensor(
                                out=astr2[:, i, :],
                                in0=jidx0[:, 0:256],
                                scalar=scol_sb[:, half, 2 * m + i : 2 * m + i + 1],
                                in1=astr2[:, i, :],
                                op0=mybir.AluOpType.is_ge,
                                op1=mybir.AluOpType.mult,
                            )
                            nc.tensor.matmul(
                                srow[:],
                                lhsT=ones_col[:],
                                rhs=astr2[:, i, :],
                                start=(m == 0 and i == 0),
                                stop=(m == NPAIR - 1 and i == 1),
                            )
                        for dc in range(KT):
                            nc.tensor.matmul(
                                pav[:, dc, :],
                                lhsT=vj2[:, :, dc * P : (dc + 1) * P],
                                rhs=astr2[:],
                                start=(m == 0),
                                stop=(m == NPAIR - 1),
                                perf_mode=DR,
                            )
                    # normalize: oT = pav * (OUT_SCALE/16) / rowsum, in fp8
                    rr = sbat.tile([1, 256], F32, tag="rr")
                    nc.vector.reciprocal(rr[:], srow[:])
                    rr2 = sbat.tile([1, 256], BF16, tag="rr2")
                    nc.vector.tensor_scalar(
                        out=rr2[:],
                        in0=rr[:],
                        scalar1=OUT_SCALE / K_SCALE,
                        scalar2=None,
                        op0=mybir.AluOpType.mult,
                    )
                    bc = ps_bc.tile([P, 256], F32, tag="bc")
                    nc.tensor.matmul(
                        bc[:], lhsT=ones_row[:], rhs=rr2[:], start=True, stop=True
                    )
                    bc_sb = sbat.tile([P, 256], F32, tag="bc_sb")
                    nc.vector.tensor_copy(out=bc_sb[:], in_=bc[:])
                    for dc in range(KT):
                        nc.vector.scalar_tensor_tensor(
                            out=oT_keep[:, dc, half * 256 : (half + 1) * 256],
                            in0=pav[:, dc, :],
                            scalar=1.0,
                            in1=bc_sb[:],
                            op0=mybir.AluOpType.mult,
                            op1=mybir.AluOpType.mult,
                        )

            # ---------------- logits: own 512 tokens x full vocab -----------
            # Wp streamed from HBM strip by strip; each strip reused for the
            # 4 own row-blocks.
            with (
                tc.tile_pool(name="sbwp", bufs=3) as sbwp,
                tc.tile_pool(name="sblo", bufs=8) as sblo,
                tc.tile_pool(name="pslg", bufs=7, space="PSUM") as pslg,
            ):
                for s in range(NSTRIP):
                    v0 = s * 512
                    wps = sbwp.tile([P, KT, 512], FP8, tag="wps")
                    nc.sync.dma_start(
                        wps[:],
                        Wp8.ap()[:, v0 : v0 + 512].rearrange(
                            "(kt p) v -> p kt v", p=P
                        ),
                    )
                    for q in range(IBLK):
                        pl = pslg.tile([P, 512], F32, tag="lg")
                        for k2 in range(KT // 2):
                            nc.tensor.matmul(
                                pl[:],
                                lhsT=oT_keep[
                                    :, 2 * k2 : 2 * k2 + 2, q * P : (q + 1) * P
                                ],
                                rhs=wps[:, 2 * k2 : 2 * k2 + 2, :],
                                start=(k2 == 0),
                                stop=(k2 == KT // 2 - 1),
                                perf_mode=DR,
                            )
                        lo = sblo.tile([P, 512], BF16, tag="lo")
                        nc.scalar.activation(
                            lo[:, :256], pl[:, :256],
                            mybir.ActivationFunctionType.Copy,
                        )
                        nc.vector.tensor_copy(out=lo[:, 256:], in_=pl[:, 256:])
                        nc.gpsimd.dma_start(
                            logits.ap()[q * P : (q + 1) * P, v0 : v0 + 512],
                            lo[:],
                        )
            oT_pool.release()
            kT_pool.release()
            qT_pool.release()
    return nc


def _prep_inputs(inputs):
    """Host-side shard prep: slicing, transposes, padding, dtype pre-casts."""
    tokens = np.ascontiguousarray(np.asarray(inputs["tokens"]).astype(np.int32))
    E16 = np.asarray(inputs["E"], np.float32).astype(ml_dtypes.bfloat16)
    WqT = np.ascontiguousarray(
        np.asarray(inputs["Wq"], np.float32).T.astype(ml_dtypes.bfloat16)
    )
    WkT = np.ascontiguousarray(
        np.asarray(inputs["Wk"], np.float32).T.astype(ml_dtypes.bfloat16)
    )
    WvT = np.ascontiguousarray(
        np.asarray(inputs["Wv"], np.float32).T.astype(ml_dtypes.bfloat16)
    )
    Wp = np.asarray(inputs["Wp"], np.float32)
    bv = np.asarray(inputs["bv"], np.float32)
    WpT_pad = np.zeros((D, VPAD), np.float32)
    WpT_pad[:, :VOCAB] = Wp.T
    Wp8_full = np.ascontiguousarray(
        (WpT_pad * WP_SCALE).astype(ml_dtypes.float8_e4m3)
    )
    bias_full = np.zeros((VPAD,), np.float32)
    bias_full[:VOCAB] = np.asarray(inputs["bp"], np.float32) + Wp @ bv

    in_maps = []
    p_idx = np.arange(P, dtype=np.float32)
    for c in range(NC):
        # tb-major key order: pair m sub i -> key tile of rank r=m%8,
        # local tile t = (m//8)*2 + i; key base token = r*512 + t*128
        scol = np.empty((2, 2 * NPAIR, P), np.float32)
        for h in range(2):
            for m in range(NPAIR):
                r = m % NC
                for i in range(2):
                    t = (m // NC) * 2 + i
                    base = r * 512 + t * 128
                    scol[h, 2 * m + i, :] = (
                        base + p_idx - (c * 512.0 + h * 256.0)
                    )
        in_maps.append(
            {
                "qtok": np.ascontiguousarray(tokens[c * OWN : (c + 1) * OWN]),
                "E": E16,
                "WqT": WqT,
                "WkT": WkT,
                "WvT": WvT,
                "bq": np.asarray(inputs["bq"], np.float32),
                "bk": np.asarray(inputs["bk"], np.float32),
                "Wp8": Wp8_full,
                "scol": np.ascontiguousarray(scol.ravel()),
            }
        )
    return in_maps, bias_full


def _run(inputs, trace=False):
    nc = bacc.Bacc(trn_type="TRN2", num_devices=NC, debug=False)
    build(nc)
    nc.compile()
    in_maps, bias_full = _prep_inputs(inputs)
    res = run_bass_kernel_spmd(
        nc, in_maps, core_ids=list(range(NC)), trace=trace
    )
    dq = 1.0 / (WP_SCALE * OUT_SCALE)
    out = np.concatenate(
        [
            np.asarray(res.results[c]["logits"], np.float32) * dq
            + bias_full[None, :]
            for c in range(NC)
        ],
        axis=0,
    )[:, :VOCAB]
    return out, res


def kernel(**inputs) -> np.ndarray:
    out, _ = _run(inputs, trace=False)
    return out
